# revision 5
# baseline (speedup 1.0000x reference)
"""CIF (Continuous Integrate-and-Fire) segment-reduce kernel for Trainium2 (8 NeuronCores).

Structure of the problem (B=32, T=2000, H=512, L_OUT=250, threshold=0.95):

  * The scan over T is a recurrence ONLY in the scalar integrator driven by
    `alphas` [B,T] (256 KB).  It never touches `hidden`.  We replicate the
    reference's sequential fp32 arithmetic exactly on the host (same op
    order -> bit-identical fire decisions), which yields, for every step t,
    at most two (output-slot, weight) contributions:
      - no fire:  alpha_t             -> slot n_prev
      - fire:     1 - integrate_{t-1} -> slot n_prev   (emitted frame's last term)
                  alpha_t - dist_comp -> slot n_prev+1 (next frame's first term)
    where n_prev = number of fires before t.  Contributions to slots that
    never get emitted (>= min(#fires, L_OUT)) are dropped, matching the
    reference's gather/valid masking.

  * The heavy part, out[b,l] = sum_t W[b,l,t] * hidden[b,t], is a banded
    matmul (band drift is exactly 15.625 slots per 125-step chunk since
    sum(alphas) == 250; deviation is a Brownian bridge, sigma <~2 slots).
    It runs on the 8 NeuronCores, data-parallel over B (4 examples/core).
    Per example the 250 output slots live in two PSUM "panels" (banks) of
    128 slots; each matmul accumulates W_tile[125,128]^T @ h_tile[125,512]
    into the panel(s) its band intersects (blocks [0,875) -> panel 0,
    [1125,2000) -> panel 1, boundary block [875,1125) -> both panels with
    disjoint column halves, which the weight builder asserts).

  * DMA strategy: hidden stays fp32 all the way (no in-flight cast -- the
    cast forced everything onto the single SWDGE queue and its engine
    balance collapsed).  The three per-example hidden blocks stream in
    parallel on the three dynamic queues (sync HWDGE / scalar HWDGE /
    gpsimd SWDGE), one DMA per (example, block) so example e's matmuls
    overlap example e+1's loads.  The PE consumes the fp32 tiles as
    `float32r` (single-pass matmul at N>=256 -- same 1 cycle/row as fp16).
    The walrus verifier forbids mixing 32-bit and 16-bit matmul operands,
    so W rides HBM as fp16 and the DVE casts it to f32r in SBUF (~1.2 us
    per example, vector engine is otherwise idle).  Output goes back as
    fp16 (host casts to fp32): halves output traffic, adds ~2e-4 rounding
    against a 2e-2 budget.

Memory traffic per core ~ 16.4 MB hidden + 2.3 MB W + 1 MB out -> memory-bound;
HBM-per-NC limit 358 GB/s => ~55 us roofline.
"""

import numpy as np

B, T, H = 32, 2000, 512
L_OUT = 250
N_CORES = 8
EX_PER_CORE = B // N_CORES      # 4
NCHUNK = 16                     # T-chunks per example
KC = T // NCHUNK                # 125 steps per chunk
LPAD = 256                      # padded slot axis (2 panels x 128)

# Hidden streams in 3 blocks per example; partition p of a block tile holds
# the S consecutive timesteps t = t0 + S*p + j, j<S (one contiguous S*2 KB
# HBM read per partition -> large DMA descriptors -> full SDMA bandwidth).
# Each matmul contracts sub-chunk j = the 125 strided steps {t0 + S*p + j};
# the weight builder permutes W rows to match, so the sum is unchanged.
# Output slots live in two PSUM panels of 128.  Slot position at step t is
# t/8 +- dev (Brownian bridge, sigma ~1.6 slots), so block [0,875) can only
# touch panel 0 and block [1125,2000) only panel 1 (11+ sigma margins,
# asserted); the boundary block [875,1125) hits both.
BLOCKS = [  # (t0, t1, S = steps per partition line, panels)
    (0, 875, 7, (0,)),
    (875, 1125, 2, (0, 1)),
    (1125, 2000, 7, (1,)),
]
MMS = [
    (bl, j, p)
    for bl, (t0, t1, S, panels) in enumerate(BLOCKS)
    for p in panels
    for j in range(S)
]
NMM = len(MMS)                  # 18

_PROGRAM = None        # cached compiled Bass program
LAST_RESULT = None     # BassKernelResults of the most recent run (introspection)
RUN_KWARGS = {}        # extra kwargs for run_bass_kernel_spmd (e.g. trace=True)


def _host_scan_weights(alphas: np.ndarray):
    """Replicates the reference scan's fp32 arithmetic exactly.

    Returns (wa, Ai, wb, Bi, ntot): per-step primary weight/slot, secondary
    (fire-only) weight/slot, and total fires per row.
    """
    a = np.ascontiguousarray(alphas, dtype=np.float32)
    Bb, Tt = a.shape
    ONE = np.float32(1.0)
    TH = np.float32(0.95)
    integrate = np.zeros(Bb, np.float32)
    n = np.zeros(Bb, np.int32)
    wa = np.empty((Bb, Tt), np.float32)
    wb = np.zeros((Bb, Tt), np.float32)
    Ai = np.empty((Bb, Tt), np.int32)
    Bi = np.empty((Bb, Tt), np.int32)
    for t in range(Tt):
        al = a[:, t]
        dist = ONE - integrate          # distribution_completion (fp32)
        integ = integrate + al          # fp32, same single add as reference
        f = integ > TH
        cur = np.where(f, dist, al)
        wa[:, t] = cur
        Ai[:, t] = n                    # n_prev
        wb[:, t] = np.where(f, al - cur, np.float32(0.0))
        Bi[:, t] = n + 1
        n = n + f
        integrate = np.where(f, integ - ONE, integ)  # exact subtract (Sterbenz)
    return wa, Ai, wb, Bi, n


def _build_weight_windows(alphas: np.ndarray) -> np.ndarray:
    """Returns W [B, KC, NMM, 128] float16 panel weight tiles."""
    wa, Ai, wb, Bi, ntot = _host_scan_weights(alphas)
    lim = np.minimum(ntot, L_OUT)[:, None].astype(np.int32)
    wa = np.where(Ai < lim, wa, np.float32(0.0))
    wb = np.where(Bi < lim, wb, np.float32(0.0))

    Wd = np.zeros((B, T, LPAD), np.float32)
    bi = np.arange(B)[:, None]
    ti = np.arange(T)[None, :]
    Wd[bi, ti, np.minimum(Bi, LPAD - 1)] = wb
    Wd[bi, ti, np.minimum(Ai, LPAD - 1)] = wa

    # panel-coverage asserts: every block's band must be inside the union of
    # the panels it is assigned to.
    for bl, (t0, t1, S, panels) in enumerate(BLOCKS):
        if 0 not in panels and Wd[:, t0:t1, :128].any():
            raise AssertionError(f"block {bl} has panel-0 mass but no panel-0 matmul")
        if 1 not in panels and Wd[:, t0:t1, 128:].any():
            raise AssertionError(f"block {bl} has panel-1 mass but no panel-1 matmul")

    W = np.empty((B, KC, NMM, 128), np.float16)
    for i, (bl, j, p) in enumerate(MMS):
        t0, t1, S, _ = BLOCKS[bl]
        # [B, p(=partition), j, slot] with t = t0 + S*p + j
        blk = Wd[:, t0:t1, :].reshape(B, KC, S, LPAD)
        W[:, :, i, :] = blk[:, :, j, p * 128 : (p + 1) * 128]
    return np.ascontiguousarray(W)


def _build_program():
    """Builds + compiles the per-core Bass/Tile program (SPMD, shared)."""
    import concourse.bacc as bacc
    import concourse.mybir as mybir
    import concourse.tile as tile

    nc = bacc.Bacc("TRN2", target_bir_lowering=False, debug=False, num_devices=N_CORES)
    f32r = mybir.dt.float32r
    f32 = mybir.dt.float32
    f16 = mybir.dt.float16

    hid = nc.dram_tensor(
        "hidden_sh", [EX_PER_CORE, T, H], f32r, kind="ExternalInput"
    )
    wwin = nc.dram_tensor(
        "w_sh", [EX_PER_CORE, KC, NMM, 128], f16, kind="ExternalInput"
    )
    out = nc.dram_tensor(
        "out_sh", [EX_PER_CORE, L_OUT, H], f16, kind="ExternalOutput"
    )

    # queue per block: the three hidden blocks ride the three dynamic DMA
    # queues in parallel; W + boundary + output share the (lighter) SWDGE.
    with tile.TileContext(nc) as tc:
        with (
            tc.tile_pool(name="hp0", bufs=EX_PER_CORE) as hpool0,
            tc.tile_pool(name="hp1", bufs=EX_PER_CORE) as hpool1,
            tc.tile_pool(name="hp2", bufs=EX_PER_CORE) as hpool2,
            tc.tile_pool(name="wp16", bufs=EX_PER_CORE) as wpool16,
            tc.tile_pool(name="wp32", bufs=EX_PER_CORE) as wpool32,
            tc.tile_pool(name="ob", bufs=4) as opool,
            tc.tile_pool(name="psp", bufs=4, space="PSUM") as pspool,
        ):
            hpools = [hpool0, hpool1, hpool2]

            htiles = []
            wtiles16 = []
            for e in range(EX_PER_CORE):
                wt16 = wpool16.tile([KC, NMM, 128], f16)
                nc.gpsimd.dma_start(wt16[:], wwin[e])
                wtiles16.append(wt16)
                row = []
                for bl, (t0, t1, S, _) in enumerate(BLOCKS):
                    hsrc = hid[e, t0:t1, :].rearrange("(p j) h -> p j h", j=S)
                    ht = hpools[bl].tile([KC, S, H], f32r, name=f"hb{bl}")
                    eng = (nc.sync, nc.gpsimd, nc.scalar)[bl]
                    eng.dma_start(ht[:], hsrc)
                    row.append(ht)
                htiles.append(row)

            for e in range(EX_PER_CORE):
                wt = wpool32.tile([KC, NMM, 128], f32r)
                nc.vector.tensor_copy(wt[:], wtiles16[e][:])
                panels = [
                    pspool.tile([128, H], f32, name=f"panel{p}", tag=f"panel{p}")
                    for p in range(2)
                ]
                first = [True, True]
                last_i = {
                    p: max(i for i, m in enumerate(MMS) if m[2] == p) for p in (0, 1)
                }
                for i, (bl, j, p) in enumerate(MMS):
                    nc.tensor.matmul(
                        panels[p][:], wt[:, i, :], htiles[e][bl][:, j, :],
                        start=first[p], stop=(i == last_i[p]),
                    )
                    first[p] = False
                ob0 = opool.tile([128, H], f16)
                nc.vector.tensor_copy(ob0[:], panels[0][:])
                nc.gpsimd.dma_start(out[e, 0:128, :], ob0[:])
                ob1 = opool.tile([128, H], f16)
                nc.vector.tensor_copy(ob1[0 : L_OUT - 128, :], panels[1][0 : L_OUT - 128, :])
                nc.gpsimd.dma_start(out[e, 128:L_OUT, :], ob1[0 : L_OUT - 128, :])
    nc.compile()
    return nc


def kernel(hidden: np.ndarray, alphas: np.ndarray) -> np.ndarray:
    global _PROGRAM, LAST_RESULT
    from concourse.bass_utils import run_bass_kernel_spmd

    hidden = np.ascontiguousarray(np.asarray(hidden), dtype=np.float32)
    alphas = np.ascontiguousarray(np.asarray(alphas), dtype=np.float32)
    assert hidden.shape == (B, T, H) and alphas.shape == (B, T)

    Wwin = _build_weight_windows(alphas)

    if _PROGRAM is None:
        _PROGRAM = _build_program()
    nc = _PROGRAM

    in_maps = [
        {
            "hidden_sh": hidden[i * EX_PER_CORE : (i + 1) * EX_PER_CORE],
            "w_sh": Wwin[i * EX_PER_CORE : (i + 1) * EX_PER_CORE],
        }
        for i in range(N_CORES)
    ]
    res = run_bass_kernel_spmd(nc, in_maps, list(range(N_CORES)), **RUN_KWARGS)
    LAST_RESULT = res
    out16 = np.concatenate([r["out_sh"] for r in res.results], axis=0)
    return out16.astype(np.float32)


# revision 6
# speedup vs baseline: 1.9704x; 1.9704x over previous
"""CIF (Continuous Integrate-and-Fire) segment-reduce kernel for Trainium2 (8 NeuronCores).

Structure of the problem (B=32, T=2000, H=512, L_OUT=250, threshold=0.95):

  * The scan over T is a recurrence ONLY in the scalar integrator driven by
    `alphas` [B,T] (256 KB).  It never touches `hidden`.  We replicate the
    reference's sequential fp32 arithmetic exactly on the host (same op
    order -> bit-identical fire decisions), which yields, for every step t,
    at most two (output-slot, weight) contributions:
      - no fire:  alpha_t             -> slot n_prev
      - fire:     1 - integrate_{t-1} -> slot n_prev   (emitted frame's last term)
                  alpha_t - dist_comp -> slot n_prev+1 (next frame's first term)
    where n_prev = number of fires before t.  Contributions to slots that
    never get emitted (>= min(#fires, L_OUT)) are dropped, matching the
    reference's gather/valid masking.

  * The heavy part, out[b,l] = sum_t W[b,l,t] * hidden[b,t], is a banded
    matmul (band drift is exactly 15.625 slots per 125-step chunk since
    sum(alphas) == 250; deviation is a Brownian bridge, sigma <~2 slots).
    It runs on the 8 NeuronCores, data-parallel over B (4 examples/core).
    Per example the 250 output slots live in two PSUM "panels" (banks) of
    128 slots; each matmul accumulates W_tile[125,128]^T @ h_tile[125,512]
    into the panel(s) its band intersects (blocks [0,875) -> panel 0,
    [1125,2000) -> panel 1, boundary block [875,1125) -> both panels with
    disjoint column halves, which the weight builder asserts).

  * DMA strategy (the whole game -- this kernel is HBM-bound).  Queue ->
    SDMA-engine mapping measured on this runtime: both HWDGE rings
    (nc.sync / nc.scalar) pin every DMA to SDMA lanes 0-4 in 5 equal
    contiguous runs; the SWDGE ring (nc.gpsimd) also splits each DMA 5
    ways but ROTATES the 5-lane window by 5 per issued DMA, so 16
    consecutive DMAs land on every lane exactly 5 times (gcd(5,16)=1).
    Therefore: hidden (fp32->fp16 cast in flight, SWDGE-only feature) and
    the fp16 weights all ride SWDGE as exactly 16 interleaved DMAs per
    core (W,b0,b1,b2 x 4 examples = one full rotation, near-even lane
    bytes); the fp16 output rides the scalar HWDGE ring whose lanes 0-4
    are idle by then.  fp16 operands keep the PE on single-pass matmuls;
    output returns as fp16 (host casts up; ~2e-4 extra rounding against a
    2e-2 budget).

Memory traffic per core ~ 16.4 MB hidden read + 2.3 MB W + 1 MB out
-> 19.7 MB vs 358 GB/s HBM-per-NC => ~55 us roofline.
"""

import numpy as np

B, T, H = 32, 2000, 512
L_OUT = 250
N_CORES = 8
EX_PER_CORE = B // N_CORES      # 4
NCHUNK = 16                     # T-chunks per example
KC = T // NCHUNK                # 125 steps per chunk
LPAD = 256                      # padded slot axis (2 panels x 128)

# Hidden streams in 3 blocks per example; partition p of a block tile holds
# the S consecutive timesteps t = t0 + S*p + j, j<S (one contiguous S*2 KB
# HBM read per partition -> large DMA descriptors -> full SDMA bandwidth).
# Each matmul contracts sub-chunk j = the 125 strided steps {t0 + S*p + j};
# the weight builder permutes W rows to match, so the sum is unchanged.
# Output slots live in two PSUM panels of 128.  Slot position at step t is
# t/8 +- dev (Brownian bridge, sigma ~1.6 slots), so block [0,875) can only
# touch panel 0 and block [1125,2000) only panel 1 (11+ sigma margins,
# asserted); the boundary block [875,1125) hits both.
BLOCKS = [  # (t0, t1, S = steps per partition line, panels)
    (0, 875, 7, (0,)),
    (875, 1125, 2, (0, 1)),
    (1125, 2000, 7, (1,)),
]
MMS = [
    (bl, j, p)
    for bl, (t0, t1, S, panels) in enumerate(BLOCKS)
    for p in panels
    for j in range(S)
]
NMM = len(MMS)                  # 18

_PROGRAM = None        # cached compiled Bass program
LAST_RESULT = None     # BassKernelResults of the most recent run (introspection)
RUN_KWARGS = {}        # extra kwargs for run_bass_kernel_spmd (e.g. trace=True)


def _host_scan_weights(alphas: np.ndarray):
    """Replicates the reference scan's fp32 arithmetic exactly.

    Returns (wa, Ai, wb, Bi, ntot): per-step primary weight/slot, secondary
    (fire-only) weight/slot, and total fires per row.
    """
    a = np.ascontiguousarray(alphas, dtype=np.float32)
    Bb, Tt = a.shape
    ONE = np.float32(1.0)
    TH = np.float32(0.95)
    integrate = np.zeros(Bb, np.float32)
    n = np.zeros(Bb, np.int32)
    wa = np.empty((Bb, Tt), np.float32)
    wb = np.zeros((Bb, Tt), np.float32)
    Ai = np.empty((Bb, Tt), np.int32)
    Bi = np.empty((Bb, Tt), np.int32)
    for t in range(Tt):
        al = a[:, t]
        dist = ONE - integrate          # distribution_completion (fp32)
        integ = integrate + al          # fp32, same single add as reference
        f = integ > TH
        cur = np.where(f, dist, al)
        wa[:, t] = cur
        Ai[:, t] = n                    # n_prev
        wb[:, t] = np.where(f, al - cur, np.float32(0.0))
        Bi[:, t] = n + 1
        n = n + f
        integrate = np.where(f, integ - ONE, integ)  # exact subtract (Sterbenz)
    return wa, Ai, wb, Bi, n


def _build_weight_windows(alphas: np.ndarray) -> np.ndarray:
    """Returns W [B, KC, NMM, 128] float16 panel weight tiles."""
    wa, Ai, wb, Bi, ntot = _host_scan_weights(alphas)
    lim = np.minimum(ntot, L_OUT)[:, None].astype(np.int32)
    wa = np.where(Ai < lim, wa, np.float32(0.0))
    wb = np.where(Bi < lim, wb, np.float32(0.0))

    Wd = np.zeros((B, T, LPAD), np.float32)
    bi = np.arange(B)[:, None]
    ti = np.arange(T)[None, :]
    Wd[bi, ti, np.minimum(Bi, LPAD - 1)] = wb
    Wd[bi, ti, np.minimum(Ai, LPAD - 1)] = wa

    # panel-coverage asserts: every block's band must be inside the union of
    # the panels it is assigned to.
    for bl, (t0, t1, S, panels) in enumerate(BLOCKS):
        if 0 not in panels and Wd[:, t0:t1, :128].any():
            raise AssertionError(f"block {bl} has panel-0 mass but no panel-0 matmul")
        if 1 not in panels and Wd[:, t0:t1, 128:].any():
            raise AssertionError(f"block {bl} has panel-1 mass but no panel-1 matmul")

    W = np.empty((B, KC, NMM, 128), np.float16)
    for i, (bl, j, p) in enumerate(MMS):
        t0, t1, S, _ = BLOCKS[bl]
        # [B, p(=partition), j, slot] with t = t0 + S*p + j
        blk = Wd[:, t0:t1, :].reshape(B, KC, S, LPAD)
        W[:, :, i, :] = blk[:, :, j, p * 128 : (p + 1) * 128]
    return np.ascontiguousarray(W)


def _build_program():
    """Builds + compiles the per-core Bass/Tile program (SPMD, shared)."""
    import concourse.bacc as bacc
    import concourse.mybir as mybir
    import concourse.tile as tile

    nc = bacc.Bacc("TRN2", target_bir_lowering=False, debug=False, num_devices=N_CORES)
    f32 = mybir.dt.float32
    f16 = mybir.dt.float16

    hid = nc.dram_tensor(
        "hidden_sh", [EX_PER_CORE, T, H], f32, kind="ExternalInput"
    )
    wwin = nc.dram_tensor(
        "w_sh", [EX_PER_CORE, KC, NMM, 128], f16, kind="ExternalInput"
    )
    out = nc.dram_tensor(
        "out_sh", [EX_PER_CORE, L_OUT, H], f16, kind="ExternalOutput"
    )

    with tile.TileContext(nc) as tc:
        with (
            tc.tile_pool(name="hp0", bufs=EX_PER_CORE) as hpool0,
            tc.tile_pool(name="hp1", bufs=EX_PER_CORE) as hpool1,
            tc.tile_pool(name="hp2", bufs=EX_PER_CORE) as hpool2,
            tc.tile_pool(name="wp", bufs=EX_PER_CORE) as wpool,
            tc.tile_pool(name="ob", bufs=4) as opool,
            tc.tile_pool(name="psp", bufs=4, space="PSUM") as pspool,
        ):
            hpools = [hpool0, hpool1, hpool2]
            # All inputs ride the rotating SWDGE queue: the W,b0,b1,b2
            # interleave x 4 examples = exactly one 16-DMA rotation period,
            # spreading bytes near-evenly over the 16 SDMA lanes.
            htiles = []
            wtiles = []
            for e in range(EX_PER_CORE):
                wt = wpool.tile([KC, NMM, 128], f16)
                nc.gpsimd.dma_start(wt[:], wwin[e])
                wtiles.append(wt)
                row = []
                for bl, (t0, t1, S, _) in enumerate(BLOCKS):
                    hsrc = hid[e, t0:t1, :].rearrange("(p j) h -> p j h", j=S)
                    ht = hpools[bl].tile([KC, S, H], f16, name=f"hb{bl}")
                    nc.gpsimd.dma_start(ht[:], hsrc)
                    row.append(ht)
                htiles.append(row)

            for e in range(EX_PER_CORE):
                wt = wtiles[e]
                panels = [
                    pspool.tile([128, H], f32, name=f"panel{p}", tag=f"panel{p}")
                    for p in range(2)
                ]
                first = [True, True]
                last_i = {
                    p: max(i for i, m in enumerate(MMS) if m[2] == p) for p in (0, 1)
                }
                for i, (bl, j, p) in enumerate(MMS):
                    nc.tensor.matmul(
                        panels[p][:], wt[:, i, :], htiles[e][bl][:, j, :],
                        start=first[p], stop=(i == last_i[p]),
                    )
                    first[p] = False
                ob0 = opool.tile([128, H], f16)
                nc.vector.tensor_copy(ob0[:], panels[0][:])
                nc.scalar.dma_start(out[e, 0:128, :], ob0[:])
                ob1 = opool.tile([128, H], f16)
                nc.vector.tensor_copy(ob1[0 : L_OUT - 128, :], panels[1][0 : L_OUT - 128, :])
                nc.scalar.dma_start(out[e, 128:L_OUT, :], ob1[0 : L_OUT - 128, :])
    nc.compile()
    return nc


def kernel(hidden: np.ndarray, alphas: np.ndarray) -> np.ndarray:
    global _PROGRAM, LAST_RESULT
    from concourse.bass_utils import run_bass_kernel_spmd

    hidden = np.ascontiguousarray(np.asarray(hidden), dtype=np.float32)
    alphas = np.ascontiguousarray(np.asarray(alphas), dtype=np.float32)
    assert hidden.shape == (B, T, H) and alphas.shape == (B, T)

    Wwin = _build_weight_windows(alphas)

    if _PROGRAM is None:
        _PROGRAM = _build_program()
    nc = _PROGRAM

    in_maps = [
        {
            "hidden_sh": hidden[i * EX_PER_CORE : (i + 1) * EX_PER_CORE],
            "w_sh": Wwin[i * EX_PER_CORE : (i + 1) * EX_PER_CORE],
        }
        for i in range(N_CORES)
    ]
    res = run_bass_kernel_spmd(nc, in_maps, list(range(N_CORES)), **RUN_KWARGS)
    LAST_RESULT = res
    out16 = np.concatenate([r["out_sh"] for r in res.results], axis=0)
    return out16.astype(np.float32)


# revision 8
# speedup vs baseline: 2.0520x; 1.0414x over previous
"""CIF (Continuous Integrate-and-Fire) segment-reduce kernel for Trainium2 (8 NeuronCores).

Structure of the problem (B=32, T=2000, H=512, L_OUT=250, threshold=0.95):

  * The scan over T is a recurrence ONLY in the scalar integrator driven by
    `alphas` [B,T] (256 KB).  It never touches `hidden`.  We replicate the
    reference's sequential fp32 arithmetic exactly on the host (same op
    order -> bit-identical fire decisions), which yields, for every step t,
    at most two (output-slot, weight) contributions:
      - no fire:  alpha_t             -> slot n_prev
      - fire:     1 - integrate_{t-1} -> slot n_prev   (emitted frame's last term)
                  alpha_t - dist_comp -> slot n_prev+1 (next frame's first term)
    where n_prev = number of fires before t.  Contributions to slots that
    never get emitted (>= min(#fires, L_OUT)) are dropped, matching the
    reference's gather/valid masking.

  * The heavy part, out[b,l] = sum_t W[b,l,t] * hidden[b,t], is a banded
    matmul (band drift is exactly 15.625 slots per 125-step chunk since
    sum(alphas) == 250; deviation is a Brownian bridge, sigma <~2 slots).
    It runs on the 8 NeuronCores, data-parallel over B (4 examples/core).
    Per example the 250 output slots live in two PSUM "panels" (banks) of
    128 slots; each matmul accumulates W_tile[125,128]^T @ h_tile[125,512]
    into the panel(s) its band intersects (blocks [0,875) -> panel 0,
    [1125,2000) -> panel 1, boundary block [875,1125) -> both panels with
    disjoint column halves, which the weight builder asserts).

  * DMA strategy (the whole game -- this kernel is HBM-bound).  Measured
    queue->SDMA-lane mapping on this runtime: both HWDGE rings pin every
    DMA to lanes 0-4; the SWDGE ring splits each DMA into 5 equal
    contiguous descriptor runs on lanes {5k..5k+4} (mod 16) where k is the
    DMA's issue index -- confirmed to 3 decimals against per-lane byte
    counts.  So everything (hidden with fp32->fp16 cast in flight, fp16 W,
    fp16 out) rides SWDGE, as 24 input DMAs whose issue order was chosen
    by brute force to minimize the max per-lane byte load (1.203 MB vs
    1.168 ideal) while staggering example readiness for the matmul
    pipeline.  b0/b2 are split into 4+3 j-halves for finer packing quanta.
    dynamic_dma_scratch_size=64KB (4x default) sizes the descriptor rings
    so all 24 issues happen up front without mid-stream stalls.

  * PE: fp16 single-pass matmuls, N=512.  The PE_HAM clock gate halves the
    PE clock unless it sees ~3.4 us of sustained activity, and re-throttles
    after ~5 us idle; zero-value dummy matmuls warm it before the first
    example and bridge inter-example gaps so the real matmuls run at 2.4
    GHz.  Panel copies go vector (panel 0) / scalar (panel 1) in parallel;
    output returns as fp16 (host casts up; ~2e-4 extra rounding against a
    2e-2 budget).

Memory traffic per core ~ 16.4 MB hidden read + 2.3 MB W read + 1 MB out
write ~= 19.7 MB vs 358 GB/s HBM-per-NC and ~390 GB/s lane-aggregate
=> ~52 us stream floor + ~8 us preamble + ~5 us tail.
"""

import numpy as np

B, T, H = 32, 2000, 512
L_OUT = 250
N_CORES = 8
EX_PER_CORE = B // N_CORES      # 4
NCHUNK = 16                     # T-chunks per example
KC = T // NCHUNK                # 125 steps per chunk
LPAD = 256                      # padded slot axis (2 panels x 128)

# Hidden streams in 3 blocks per example; partition p of a block tile holds
# the S consecutive timesteps t = t0 + S*p + j, j<S (one contiguous S*2 KB
# HBM read per partition).  Each matmul contracts sub-chunk j = the 125
# strided steps {t0 + S*p + j}; the weight builder permutes W rows to
# match, so the sum is unchanged.  Slot position at step t is t/8 +- dev
# (Brownian bridge, sigma ~1.6 slots), so block [0,875) can only touch
# panel 0 and block [1125,2000) only panel 1 (11+ sigma margins, asserted);
# the boundary block [875,1125) hits both.
BLOCKS = [  # (t0, t1, S = steps per partition line, panels)
    (0, 875, 7, (0,)),
    (875, 1125, 2, (0, 1)),
    (1125, 2000, 7, (1,)),
]
# Matmul order: panel-0 b0 block, panel-1 b2 block, then the small boundary
# matmuls close both panels -- so when the boundary tile is the last DMA to
# land, only 4 short matmuls + the copies remain on the critical tail.
MMS = (
    [(0, j, 0) for j in range(7)]
    + [(2, j, 1) for j in range(7)]
    + [(1, 0, 0), (1, 1, 0), (1, 0, 1), (1, 1, 1)]
)
NMM = len(MMS)                  # 18

# Input sub-transfers: (block, j0, j1); b0/b2 ride as 4+3 j-halves.
SUBS = {
    "W": None,
    "b0a": (0, 0, 4),
    "b0b": (0, 4, 7),
    "b1": (1, 0, 2),
    "b2a": (2, 0, 4),
    "b2b": (2, 4, 7),
}
# Issue order of the 24 input DMAs (6 per example, example-grouped).  Slot
# k lands on SDMA lanes {5k..5k+4} mod 16; this order gives max-lane load
# 1.203 MB (ideal 1.168) and puts each example's boundary tile last-ish so
# the tail chain after the final landing is just 4 matmuls + copies.
SLOT_PLAN = [
    ["b0b", "b0a", "b1", "b2b", "W", "b2a"],
    ["W", "b1", "b0a", "b0b", "b2b", "b2a"],
    ["b0a", "b2a", "b0b", "b2b", "W", "b1"],
    ["b0a", "b0b", "b2a", "W", "b2b", "b1"],
]

N_WARM = 10   # dummy matmuls before the first example (HAM warm-up)
N_KEEP = 8    # dummy matmuls between examples (bridge idle < ~5 us window)

_PROGRAM = None        # cached compiled Bass program
LAST_RESULT = None     # BassKernelResults of the most recent run (introspection)
RUN_KWARGS = {}        # extra kwargs for run_bass_kernel_spmd (e.g. trace=True)


def _host_scan_weights(alphas: np.ndarray):
    """Replicates the reference scan's fp32 arithmetic exactly.

    Returns (wa, Ai, wb, Bi, ntot): per-step primary weight/slot, secondary
    (fire-only) weight/slot, and total fires per row.
    """
    a = np.ascontiguousarray(alphas, dtype=np.float32)
    Bb, Tt = a.shape
    ONE = np.float32(1.0)
    TH = np.float32(0.95)
    integrate = np.zeros(Bb, np.float32)
    n = np.zeros(Bb, np.int32)
    wa = np.empty((Bb, Tt), np.float32)
    wb = np.zeros((Bb, Tt), np.float32)
    Ai = np.empty((Bb, Tt), np.int32)
    Bi = np.empty((Bb, Tt), np.int32)
    for t in range(Tt):
        al = a[:, t]
        dist = ONE - integrate          # distribution_completion (fp32)
        integ = integrate + al          # fp32, same single add as reference
        f = integ > TH
        cur = np.where(f, dist, al)
        wa[:, t] = cur
        Ai[:, t] = n                    # n_prev
        wb[:, t] = np.where(f, al - cur, np.float32(0.0))
        Bi[:, t] = n + 1
        n = n + f
        integrate = np.where(f, integ - ONE, integ)  # exact subtract (Sterbenz)
    return wa, Ai, wb, Bi, n


def _build_weight_windows(alphas: np.ndarray) -> np.ndarray:
    """Returns W [B, KC, NMM, 128] float16 panel weight tiles."""
    wa, Ai, wb, Bi, ntot = _host_scan_weights(alphas)
    lim = np.minimum(ntot, L_OUT)[:, None].astype(np.int32)
    wa = np.where(Ai < lim, wa, np.float32(0.0))
    wb = np.where(Bi < lim, wb, np.float32(0.0))

    Wd = np.zeros((B, T, LPAD), np.float32)
    bi = np.arange(B)[:, None]
    ti = np.arange(T)[None, :]
    Wd[bi, ti, np.minimum(Bi, LPAD - 1)] = wb
    Wd[bi, ti, np.minimum(Ai, LPAD - 1)] = wa

    # panel-coverage asserts: every block's band must be inside the union of
    # the panels it is assigned to.
    for bl, (t0, t1, S, panels) in enumerate(BLOCKS):
        if 0 not in panels and Wd[:, t0:t1, :128].any():
            raise AssertionError(f"block {bl} has panel-0 mass but no panel-0 matmul")
        if 1 not in panels and Wd[:, t0:t1, 128:].any():
            raise AssertionError(f"block {bl} has panel-1 mass but no panel-1 matmul")

    W = np.empty((B, KC, NMM, 128), np.float16)
    for i, (bl, j, p) in enumerate(MMS):
        t0, t1, S, _ = BLOCKS[bl]
        # [B, p(=partition), j, slot] with t = t0 + S*p + j
        blk = Wd[:, t0:t1, :].reshape(B, KC, S, LPAD)
        W[:, :, i, :] = blk[:, :, j, p * 128 : (p + 1) * 128]
    return np.ascontiguousarray(W)


def _build_program():
    """Builds + compiles the per-core Bass/Tile program (SPMD, shared)."""
    import concourse.bacc as bacc
    import concourse.mybir as mybir
    import concourse.tile as tile

    nc = bacc.Bacc(
        "TRN2",
        target_bir_lowering=False,
        debug=False,
        num_devices=N_CORES,
        dynamic_dma_scratch_size=65536,
    )
    f32 = mybir.dt.float32
    f16 = mybir.dt.float16

    hid = nc.dram_tensor(
        "hidden_sh", [EX_PER_CORE, T, H], f32, kind="ExternalInput"
    )
    wwin = nc.dram_tensor(
        "w_sh", [EX_PER_CORE, KC, NMM, 128], f16, kind="ExternalInput"
    )
    out = nc.dram_tensor(
        "out_sh", [EX_PER_CORE, L_OUT, H], f16, kind="ExternalOutput"
    )

    with tile.TileContext(nc) as tc:
        with (
            tc.tile_pool(name="hpa", bufs=8) as hpool_a,   # b0a/b2a [125,4,H]
            tc.tile_pool(name="hpb", bufs=8) as hpool_b,   # b0b/b2b [125,3,H]
            tc.tile_pool(name="hpc", bufs=4) as hpool_c,   # b1 [125,2,H]
            tc.tile_pool(name="wp", bufs=4) as wpool,
            tc.tile_pool(name="dummy", bufs=2) as dpool,
            tc.tile_pool(name="ob", bufs=8) as opool,
            tc.tile_pool(name="psp", bufs=3, space="PSUM") as pspool,
            tc.tile_pool(name="pspd", bufs=1, space="PSUM") as pspool_d,
        ):
            # HAM warm-up fodder: zeroed operands, dedicated PSUM bank.
            dw = dpool.tile([KC, 128], f16)
            drh = dpool.tile([KC, H], f16)
            nc.vector.memset(dw[:], 0.0)
            nc.vector.memset(drh[:], 0.0)
            dps = pspool_d.tile([128, H], f32, tag="dummy")

            def dummy_mms(n):
                for _ in range(n):
                    nc.tensor.matmul(dps[:], dw[:], drh[:], start=True, stop=True)

            # ---- input DMAs, in exact rotation slot order ----
            htiles = [dict() for _ in range(EX_PER_CORE)]
            wtiles = [None] * EX_PER_CORE
            for e in range(EX_PER_CORE):
                for kind in SLOT_PLAN[e]:
                    if kind == "W":
                        wt = wpool.tile([KC, NMM, 128], f16)
                        nc.gpsimd.dma_start(wt[:], wwin[e])
                        wtiles[e] = wt
                    else:
                        bl, j0, j1 = SUBS[kind]
                        t0, t1, S, _ = BLOCKS[bl]
                        pool = {4: hpool_a, 3: hpool_b, 2: hpool_c}[j1 - j0]
                        ht = pool.tile([KC, j1 - j0, H], f16, name=kind)
                        src = hid[e, t0:t1, :].rearrange("(p j) h -> p j h", j=S)
                        nc.gpsimd.dma_start(ht[:], src[:, j0:j1, :])
                        htiles[e][kind] = ht

            def rhs(e, bl, j):
                if bl == 1:
                    return htiles[e]["b1"][:, j, :]
                a, b = ("b0a", "b0b") if bl == 0 else ("b2a", "b2b")
                return (
                    htiles[e][a][:, j, :] if j < 4 else htiles[e][b][:, j - 4, :]
                )

            # ---- matmul + copy-out pipeline ----
            last_i = {p: max(i for i, m in enumerate(MMS) if m[2] == p) for p in (0, 1)}
            dummy_mms(N_WARM)
            for e in range(EX_PER_CORE):
                if e:
                    dummy_mms(N_KEEP)
                wt = wtiles[e]
                panels = [
                    pspool.tile([128, H], f32, name=f"panel{p}", tag=f"panel{p}")
                    for p in range(2)
                ]
                first = [True, True]
                for i, (bl, j, p) in enumerate(MMS):
                    nc.tensor.matmul(
                        panels[p][:], wt[:, i, :], rhs(e, bl, j),
                        start=first[p], stop=(i == last_i[p]),
                    )
                    first[p] = False
                ob0 = opool.tile([128, H], f16)
                nc.vector.tensor_copy(ob0[:], panels[0][:])
                nc.gpsimd.dma_start(out[e, 0:128, :], ob0[:])
                ob1 = opool.tile([128, H], f16)
                nc.scalar.copy(ob1[0 : L_OUT - 128, :], panels[1][0 : L_OUT - 128, :])
                nc.gpsimd.dma_start(out[e, 128:L_OUT, :], ob1[0 : L_OUT - 128, :])
    nc.compile()
    return nc


def kernel(hidden: np.ndarray, alphas: np.ndarray) -> np.ndarray:
    global _PROGRAM, LAST_RESULT
    from concourse.bass_utils import run_bass_kernel_spmd

    hidden = np.ascontiguousarray(np.asarray(hidden), dtype=np.float32)
    alphas = np.ascontiguousarray(np.asarray(alphas), dtype=np.float32)
    assert hidden.shape == (B, T, H) and alphas.shape == (B, T)

    Wwin = _build_weight_windows(alphas)

    if _PROGRAM is None:
        _PROGRAM = _build_program()
    nc = _PROGRAM

    in_maps = [
        {
            "hidden_sh": hidden[i * EX_PER_CORE : (i + 1) * EX_PER_CORE],
            "w_sh": Wwin[i * EX_PER_CORE : (i + 1) * EX_PER_CORE],
        }
        for i in range(N_CORES)
    ]
    res = run_bass_kernel_spmd(nc, in_maps, list(range(N_CORES)), **RUN_KWARGS)
    LAST_RESULT = res
    out16 = np.concatenate([r["out_sh"] for r in res.results], axis=0)
    return out16.astype(np.float32)
